# revision 19
# baseline (speedup 1.0000x reference)
"""ContextQueryAttention (BiDAF-style) Trainium2 kernel, v7.

Problem: nn_ContextQueryAttention_44066364457466
  query [B=8, Q=512, D=512], context [B=8, C=2048, D=512],
  query_weights/context_weights [D,1], dot_weights [D,D], mask all-True.
  out [B, C, 4D]: concat(context, c2q@query, context*that, context*qtc)

Sharding: data-parallel over batch. B == 8 == n_cores, one batch element
per NeuronCore, no collectives.

Math (per batch element; mask all-ones so it drops out):
  H[d,q]  = sum_e W[d,e] qT[e,q]       (contract the small side first:
  sim[c,q]= sum_d ctx[c,d] H[d,q] + cw[c] + qw[q]   saves 400M MACs vs
                                                    the (ctx@W)@qT order)
  Two-pass softmax around one global shift K = max sim:
    pass 1: evict raw sim to SBUF (f32), row maxes m_c on the fly
    K = max_c m_c  (tiny)
    pass 2: F = exp(sim - K + 60), row sums s_col via the accumulator.
    The +60 keeps the worst row sum (~e^{-90+60}) far enough above the
    f32 floor that 1/s_col cannot overflow; the shift cancels in both
    softmax normalizations.
  Both softmaxes come from this single F:
    c2q = F / rowsum(F)      q2c = F / colsum(F)
  ctq = c2q @ query;  G[q,d] = q2c^T @ ctx;  qtc = c2q @ G

Precision split: the sim chain (transposes, H, sim) runs f32r - exp is
exponentially sensitive to absolute logit error. Everything downstream
of exp works on attention weights where bf16's 0.4% relative error
washes out across 512-2048-term reductions: F, ET, G, and the
ctx/query copies feeding those matmuls are bf16 (the PE rejects
mixed-dtype operands, so rhs-side bf16 casts are made once per body).

Pipelined emission: each body's w/q/cw/qw loads are EMITTED midway
through the previous body (and ctx loads at its end), with a prologue
before the repeat loop, so on the input DMA queue they sit ahead of
the previous body's remaining traffic and their data is resident when
the PE crosses the body boundary. The rot8 pool cycles [wT, qT, H,
w(next)] per body - an even 64 allocations per 16-body loop iteration,
so slot phase is loop-invariant.

Other implementation notes:
- f32r via bitcast for the sim-side matmuls (PE rounds fp32 on ingest).
- Four 128x128 transposes are packed per PSUM bank, one wide eviction.
- cw enters sim through the H eviction bias; qw through one K=1 rank-1
  matmul per c-tile; S_q comes from DVE reduces of the evicted ET.
- DMA queues: sync = inputs + context output copies (dependency-free),
  scalar = ctq block (self-ordered behind its producing mul),
  gpsimd = the two ctx* blocks.
- G has its own pool so next-body wT transposes reuse H's slot (freed
  mid-body) instead of G's (freed only at body end).
- Constants are emitted once, outside the repeat loop.
"""

import numpy as np

B, Q, C, D = 8, 512, 2048, 512
P = 128
QT, CT, DT = Q // P, C // P, D // P  # 4, 16, 4
N_CORES = 8

_NC_CACHE = {}


def ds(start, size):
    return slice(start, start + size)


def _emit_consts(nc, constp):
    import concourse.mybir as mybir

    f32 = mybir.dt.float32
    f32r = mybir.dt.float32r
    bf16 = mybir.dt.bfloat16

    id_f = constp.tile([P, P], f32, name="id_f", tag="id_f")
    from concourse.masks import make_identity
    make_identity(nc, id_f)
    id_r = constp.tile([P, P], f32r, name="id_r", tag="id_r")
    nc.vector.tensor_copy(id_r, id_f)
    id_b = constp.tile([P, P], bf16, name="id_b", tag="id_b")
    nc.vector.tensor_copy(id_b, id_f)
    ones2_f = constp.tile([1, 2], f32, name="ones2_f", tag="ones2_f")
    nc.vector.memset(ones2_f, 1.0)
    ones_row_f = constp.tile([1, 512], f32, name="ones_row_f", tag="ones_row_f")
    nc.vector.memset(ones_row_f, 1.0)
    ones_row_r = constp.tile([1, 512], f32r, name="ones_row", tag="ones_row")
    nc.vector.tensor_copy(ones_row_r, ones_row_f)
    return (id_f, id_r, id_b, ones2_f, ones_row_r)


def _emit_wq_loads(nc, pools, aps):
    """Loads for ONE body's small inputs (w, q, cw, qw). Emitted midway
    through the PREVIOUS body so the data is resident at its start."""
    import concourse.mybir as mybir

    f32r = mybir.dt.float32r
    (statp, ctxps, cbfp, qfam, qbfp, rot8, gpool, fpool, simp, cTp, ETp,
     stagep, ps_mm, ps_tr, ps_st) = pools
    (q_r3, c_r3, w_r3, cw_r3, qw_r3, out_r3) = aps

    w_r = rot8.tile([P, DT, D], f32r, name="w_f", tag="r8")
    nc.sync.dma_start(w_r, w_r3)
    q_r = qfam.tile([P, QT, D], f32r, name="q_f", tag="q_f")
    nc.sync.dma_start(q_r, q_r3)
    cwqw_r = statp.tile([P, DT, 2], f32r, name="cwqw_f", tag="cwqw_f")
    nc.sync.dma_start(cwqw_r[:, :, 0:1], cw_r3)
    nc.sync.dma_start(cwqw_r[:, :, 1:2], qw_r3)
    return {"w_r": w_r, "q_r": q_r, "cwqw_r": cwqw_r}


def _emit_ctx_load(nc, pools, aps):
    """One body's ctx load; emitted at the END of the previous body
    (its dispatch is WAR-blocked on that body's last ctx readers)."""
    import concourse.mybir as mybir

    f32r = mybir.dt.float32r
    (statp, ctxps, cbfp, qfam, qbfp, rot8, gpool, fpool, simp, cTp, ETp,
     stagep, ps_mm, ps_tr, ps_st) = pools
    (q_r3, c_r3, w_r3, cw_r3, qw_r3, out_r3) = aps

    ctx_r = ctxps[0].tile([P, CT, D], f32r, name="ctx_f", tag="ctx_f")
    for g in range(4):
        nc.sync.dma_start(ctx_r[:, ds(g * 4, 4), :], c_r3[:, ds(g * 4, 4), :])
    return ctx_r


def _emit_body(nc, tc, pools, aps, consts, loads, ctx_r, emit_next=True):
    """One body. `loads`/`ctx_r` were emitted during the previous body
    (or are None for the first body of a loop iteration, which loads its
    own). Emits the NEXT body's loads at interleave points unless
    emit_next is False (last body of an iteration)."""
    import concourse.mybir as mybir

    f32 = mybir.dt.float32
    f32r = mybir.dt.float32r
    bf16 = mybir.dt.bfloat16
    Exp = mybir.ActivationFunctionType.Exp
    Copy = mybir.ActivationFunctionType.Copy
    Mult = mybir.AluOpType.mult
    Max = mybir.AluOpType.max
    Min = mybir.AluOpType.min
    AxX = mybir.AxisListType.X

    (statp, ctxps, cbfp, qfam, qbfp, rot8, gpool, fpool, simp, cTp, ETp,
     stagep, ps_mm, ps_tr, ps_st) = pools
    (q_r3, c_r3, w_r3, cw_r3, qw_r3, out_r3) = aps
    (id_f, id_r, id_b, ones2_f, ones_row_r) = consts

    if loads is None:
        loads = _emit_wq_loads(nc, pools, aps)
    if ctx_r is None:
        ctx_r = _emit_ctx_load(nc, pools, aps)
    w_r = loads["w_r"]
    q_r = loads["q_r"]
    cwqw_r = loads["cwqw_r"]
    cwqw_f = cwqw_r.bitcast(f32)
    ctx_f = ctx_r.bitcast(f32)

    # bf16 copies feeding the post-softmax matmul rhs sides
    q_bf = qbfp.tile([P, QT, D], bf16, name="q_bf", tag="q_bf")
    nc.vector.tensor_copy(q_bf, q_r.bitcast(f32))
    ctx_bf = cbfp.tile([P, CT, D], bf16, name="ctx_bf", tag="ctx_bf")
    for g in range(4):
        nc.vector.tensor_copy(ctx_bf[:, ds(g * 4, 4), :],
                              ctx_f[:, ds(g * 4, 4), :])

    def packed_transpose(dst_ap, srcs, evict_engine, ident=id_r, psdt=f32r):
        """len(srcs) transposes into one PSUM bank, one wide eviction."""
        n = len(srcs)
        ps = ps_tr.tile([P, 512], psdt, name="ptr", tag="tr")
        for j, src in enumerate(srcs):
            nc.tensor.matmul(ps[:, ds(j * P, P)], src, ident,
                             is_transpose=True, start=(j == 0),
                             stop=(j == n - 1))
        if evict_engine == "act":
            nc.scalar.copy(dst_ap, ps[:, 0: n * P])
        else:
            nc.vector.tensor_copy(dst_ap, ps[:, 0: n * P])

    # ---- wT [e,d] and qT [e,q] ----
    wT = rot8.tile([P, DT, D], f32r, name="wT", tag="r8")
    for eb in range(DT):
        packed_transpose(
            wT[:, eb, :],
            [w_r[:, dt, ds(eb * P, P)] for dt in range(DT)], "act")
    qT = rot8.tile([P, DT, Q], f32r, name="qT", tag="r8")
    for eb in range(DT):
        packed_transpose(
            qT[:, eb, :],
            [q_r[:, qt, ds(eb * P, P)] for qt in range(QT)], "act")

    # ---- H[d,q] = sum_e wT[e,d] qT[e,q]  (the small D x Q product) ----
    H = rot8.tile([P, DT, Q], f32r, name="H", tag="r8")
    for dtile in range(DT):
        pm = ps_mm.tile([P, 512], f32, name="pm", tag="mm")
        for eb in range(DT):
            nc.tensor.matmul(pm, wT[:, eb, ds(dtile * P, P)], qT[:, eb, :],
                             start=(eb == 0), stop=(eb == DT - 1))
        # H' = H + cw_w[d]: folds the cw[c] similarity term into the
        # contraction (sum_d ctx[c,d] cw_w[d] = cw[c]) at zero extra cost
        nc.scalar.activation(H[:, dtile, :], pm,
                             mybir.ActivationFunctionType.Identity,
                             bias=cwqw_f[:, dtile, 0:1], scale=1.0)

    # ---- qw_row [1, Q] ----
    qw_row = statp.tile([1, Q], f32r, name="qw_row", tag="qw_row")
    pqw = ps_st.tile([1, Q], f32, name="pst", tag="st")
    for dt in range(DT):
        nc.tensor.matmul(pqw, cwqw_r[:, dt, 1:2], qT[:, dt, :],
                         start=(dt == 0), stop=(dt == DT - 1))
    nc.vector.tensor_copy(qw_row, pqw)

    # ---- per ctx chunk: cT transposes, then raw sim for its cts ----
    cT = cTp.tile([P, DT, C], f32r, name="cT", tag="cT")
    sim_r = simp.tile([P, CT, Q], f32, name="sim_t", tag="sim_t")
    F_t = fpool.tile([P, CT, Q], bf16, name="F_t", tag="F_t")
    negm = statp.tile([P, CT], f32, name="negm", tag="negm")
    s_col = statp.tile([P, CT], f32, name="s_col", tag="s_col")

    def emit_cT_chunk(g):
        for dt in range(DT):
            packed_transpose(
                cT[:, dt, ds(g * 512, 512)],
                [ctx_r[:, 4 * g + j, ds(dt * P, P)] for j in range(4)], "dve")

    def emit_sim_chunk(g):
        # the exact-context output block goes straight from SBUF, one
        # dependency-free chunk-sized DMA instead of four per-tile ones
        nc.sync.dma_start(out_r3[:, ds(4 * g, 4), 0:D],
                          ctx_f[:, ds(4 * g, 4), :])
        for ct in range(4 * g, 4 * g + 4):
            pm = ps_mm.tile([P, 512], f32, name="pm", tag="mm")
            for dt in range(DT):
                nc.tensor.matmul(pm, cT[:, dt, ds(ct * P, P)], H[:, dt, :],
                                 start=(dt == 0), stop=False)
            nc.tensor.matmul(pm, ones_row_r[0:1, 0:P], qw_row[0:1, :],
                             start=False, stop=True)
            nc.vector.tensor_reduce(negm[:, ds(ct, 1)], pm, axis=AxX,
                                    op=Max, negate=True)
            nc.scalar.copy(sim_r[:, ct, :], pm)

    # transposes of chunk g+1 are emitted before sim of chunk g so the PE
    # never catches up with the ctx input DMA
    emit_cT_chunk(0)
    emit_cT_chunk(1)
    emit_sim_chunk(0)
    # next body's small loads: the sync queue has drained this body's
    # ctx loads + early ctx-outs by here, and the rot8/qfam slots these
    # overwrite (previous qT / q) are already consumed
    next_loads = _emit_wq_loads(nc, pools, aps) if emit_next else None
    emit_cT_chunk(2)
    emit_sim_chunk(1)
    emit_cT_chunk(3)
    emit_sim_chunk(2)
    emit_sim_chunk(3)

    # ---- global max K; negK_col = (60 - K) broadcast down partitions ----
    nkp = statp.tile([P, 1], f32, name="nkp", tag="nkp")
    nc.vector.tensor_reduce(nkp, negm, axis=AxX, op=Min)  # -max_q per row
    pkt = ps_st.tile([1, P], f32, name="pst", tag="st")
    nc.tensor.transpose(pkt, nkp, id_f)
    negK11 = statp.tile([1, 1], f32, name="negK11", tag="negK11")
    nc.vector.tensor_reduce(negK11, pkt, axis=AxX, op=Min)  # -K
    # shift by +60: F = exp(sim - K + 60). The shift cancels in both
    # softmax normalizations but keeps the worst row sum (~e^{-90+60})
    # far from the f32 range floor, so 1/s_col cannot overflow to inf.
    krow = statp.tile([1, P], f32, name="krow", tag="krow")
    nc.vector.tensor_scalar_add(krow, negK11.to_broadcast([1, P]), 60.0)
    pkb = ps_st.tile([P, 2], f32, name="pst", tag="st")
    nc.tensor.matmul(pkb, krow, ones2_f, start=True, stop=True)
    negK_col = statp.tile([P, 1], f32, name="negK_col", tag="negK_col")
    nc.vector.tensor_copy(negK_col, pkb[:, 0:1])

    # ---- F = exp(sim - K + 60) (bf16); ET [q,c] = F^T; S_q by DVE
    # reduces of the evicted ET. exp of chunk g and ET of chunk g are
    # interleaved so the PE restarts on the first four F tiles. ----
    ET = ETp.tile([P, QT, C], bf16, name="ET", tag="ET")
    sqp = statp.tile([P, QT, 4], f32, name="sqp", tag="sqp")
    for g in range(4):
        for ct in range(4 * g, 4 * g + 4):
            # scale=1.0 only: negative activation scale miscomputes on HW
            nc.scalar.activation(F_t[:, ct, :], sim_r[:, ct, :], Exp,
                                 bias=negK_col, scale=1.0,
                                 accum_out=s_col[:, ds(ct, 1)])
        for qt in range(QT):
            packed_transpose(
                ET[:, qt, ds(g * 512, 512)],
                [F_t[:, 4 * g + j, ds(qt * P, P)] for j in range(4)],
                "dve", ident=id_b, psdt=bf16)
            nc.vector.tensor_reduce(sqp[:, qt, ds(g, 1)],
                                    ET[:, qt, ds(g * 512, 512)], axis=AxX,
                                    op=mybir.AluOpType.add)
    r_col = statp.tile([P, CT], f32, name="r_col", tag="r_col")
    nc.vector.reciprocal(r_col, s_col)
    sq = statp.tile([P, QT], f32, name="sq", tag="sq")
    for qt in range(QT):
        nc.vector.tensor_reduce(sq[:, ds(qt, 1)], sqp[:, qt, :], axis=AxX,
                                op=mybir.AluOpType.add)
    rq = statp.tile([P, QT], f32, name="rq", tag="rq")
    nc.vector.reciprocal(rq, sq)

    # ---- G[q,d] = (1/S_q) sum_c F[c,q] ctx[c,d] ----
    G_b = gpool.tile([P, QT, D], bf16, name="G_b", tag="G_b")
    for qt in range(QT):
        pm = ps_mm.tile([P, 512], f32, name="pm", tag="mm")
        for ct in range(CT):
            nc.tensor.matmul(pm, F_t[:, ct, ds(qt * P, P)],
                             ctx_bf[:, ct, :],
                             start=(ct == 0), stop=(ct == CT - 1))
        nc.scalar.mul(G_b[:, qt, :], pm, rq[:, ds(qt, 1)])

    # ---- outputs per ct ----
    for ct in range(CT):
        pc = ps_mm.tile([P, 512], f32, name="pm", tag="mm")
        for qt in range(QT):
            nc.tensor.matmul(pc, ET[:, qt, ds(ct * P, P)], q_bf[:, qt, :],
                             start=(qt == 0), stop=(qt == QT - 1))
        pq = ps_mm.tile([P, 512], f32, name="pm", tag="mm")
        for qt in range(QT):
            nc.tensor.matmul(pq, ET[:, qt, ds(ct * P, P)], G_b[:, qt, :],
                             start=(qt == 0), stop=(qt == QT - 1))
        st = stagep.tile([P, 3 * D], f32, name="st", tag="stage")
        # three writers on independent engines; scalar DMAs the block it
        # produced itself (self-ordered), gpsimd takes the DVE blocks
        nc.scalar.mul(st[:, 0:D], pc, r_col[:, ds(ct, 1)])
        nc.scalar.dma_start(out_r3[:, ct, ds(D, D)], st[:, 0:D])
        ctx_slice = ctx_f[:, ct, :]
        nc.vector.scalar_tensor_tensor(st[:, ds(D, D)], pc,
                                       r_col[:, ds(ct, 1)], ctx_slice,
                                       op0=Mult, op1=Mult)
        nc.vector.scalar_tensor_tensor(st[:, ds(2 * D, D)], pq,
                                       r_col[:, ds(ct, 1)], ctx_slice,
                                       op0=Mult, op1=Mult)
        nc.gpsimd.dma_start(out_r3[:, ct, ds(2 * D, 2 * D)], st[:, ds(D, 2 * D)])

    # next body's ctx load: emitted last - its dispatch is WAR-blocked on
    # this body's final ctx_f readers (the scalar_tensor_tensor ops above)
    next_ctx = _emit_ctx_load(nc, pools, aps) if emit_next else None
    return next_loads, next_ctx


def _build_bass(loop_n=1):
    import concourse.bass as bass  # noqa: F401
    import concourse.mybir as mybir
    import concourse.tile as tile
    from concourse import bacc

    f32 = mybir.dt.float32

    f32r = mybir.dt.float32r
    nc = bacc.Bacc("TRN2", debug=False, num_devices=N_CORES)
    q_d = nc.dram_tensor("query", [Q, D], f32r, kind="ExternalInput")
    c_d = nc.dram_tensor("context", [C, D], f32r, kind="ExternalInput")
    qw_d = nc.dram_tensor("query_weights", [D, 1], f32r, kind="ExternalInput")
    cw_d = nc.dram_tensor("context_weights", [D, 1], f32r, kind="ExternalInput")
    w_d = nc.dram_tensor("dot_weights", [D, D], f32r, kind="ExternalInput")
    out_d = nc.dram_tensor("out", [C, 4 * D], f32, kind="ExternalOutput")

    aps = (
        q_d.ap().rearrange("(t p) d -> p t d", p=P),
        c_d.ap().rearrange("(t p) d -> p t d", p=P),
        w_d.ap().rearrange("(t p) e -> p t e", p=P),
        cw_d.ap().rearrange("(t p) o -> p t o", p=P),
        qw_d.ap().rearrange("(t p) o -> p t o", p=P),
        out_d.ap().rearrange("(t p) f -> p t f", p=P),
    )

    with tile.TileContext(nc) as tc:
        from contextlib import ExitStack
        with ExitStack() as es:
            def pool(name, bufs, space="SBUF"):
                return es.enter_context(
                    tc.tile_pool(name=name, bufs=bufs, space=space))

            constp = pool("const", 1)
            statp = pool("stats", 1)
            ctx0p = pool("ctx0", 1)
            ctx1p = pool("ctx1", 1)
            ctx2p = pool("ctx2", 1)
            ctx3p = pool("ctx3", 1)
            cbfp = pool("cbf", 1)
            qfam = pool("qfam", 1)
            qbfp = pool("qbf", 1)
            rot8 = pool("rot8", 2)
            gpool = pool("gpool", 2)
            fpool = pool("fpool", 1)
            simp = pool("simp", 1)
            cTp = pool("cTp", 1)
            ETp = pool("ETp", 1)
            stagep = pool("stage", 3)
            ps_mm = pool("ps_mm", 4, space="PSUM")
            ps_tr = pool("ps_tr", 3, space="PSUM")
            ps_st = pool("ps_st", 1, space="PSUM")

            pools = (statp, (ctx0p, ctx1p, ctx2p, ctx3p), cbfp, qfam,
                     qbfp, rot8, gpool, fpool, simp, cTp, ETp, stagep,
                     ps_mm, ps_tr, ps_st)
            consts = _emit_consts(nc, constp)
            if loop_n > 1:
                # unroll several bodies per loop iteration: the For_i
                # all-engine barrier is expensive on this runtime, so
                # amortize it while keeping exactly loop_n body runs.
                # Body 0 of each iteration loads its own inputs (tiles
                # allocated outside For_i are never released inside it,
                # so a prologue would deadlock the pool rotation).
                k = 16 if loop_n % 16 == 0 else (
                    8 if loop_n % 8 == 0 else (4 if loop_n % 4 == 0 else 1))
                with tc.For_i(0, loop_n // k, 1):
                    loads = ctx_r = None
                    for b in range(k):
                        loads, ctx_r = _emit_body(nc, tc, pools, aps,
                                                  consts, loads, ctx_r,
                                                  emit_next=(b < k - 1))
            else:
                _emit_body(nc, tc, pools, aps, consts, None, None,
                           emit_next=False)
    nc.compile()
    return nc


def get_nc(loop_n=1):
    if loop_n not in _NC_CACHE:
        _NC_CACHE[loop_n] = _build_bass(loop_n)
    return _NC_CACHE[loop_n]


def kernel(query, context, query_weights, context_weights, dot_weights,
           mask=None):
    from concourse.bass_utils import run_bass_kernel_spmd

    query = np.ascontiguousarray(np.asarray(query, dtype=np.float32))
    context = np.ascontiguousarray(np.asarray(context, dtype=np.float32))
    query_weights = np.ascontiguousarray(np.asarray(query_weights, dtype=np.float32))
    context_weights = np.ascontiguousarray(np.asarray(context_weights, dtype=np.float32))
    dot_weights = np.ascontiguousarray(np.asarray(dot_weights, dtype=np.float32))
    # mask is all-True per the problem spec; NEG_INF * (~mask) == 0, so it
    # drops out of the computation entirely.

    nc = get_nc()
    in_maps = [
        {
            "query": query[b],
            "context": context[b],
            "query_weights": query_weights,
            "context_weights": context_weights,
            "dot_weights": dot_weights,
        }
        for b in range(B)
    ]
    res = run_bass_kernel_spmd(nc, in_maps, core_ids=list(range(N_CORES)))
    out = np.stack([res.results[b]["out"] for b in range(B)], axis=0)
    return np.ascontiguousarray(out.astype(np.float32))


if __name__ == "__main__":
    rng = np.random.default_rng(0)
    inputs = {
        "query": rng.standard_normal((B, Q, D), dtype=np.float32),
        "context": rng.standard_normal((B, C, D), dtype=np.float32),
        "query_weights": rng.standard_normal((D, 1), dtype=np.float32) * 0.05,
        "context_weights": rng.standard_normal((D, 1), dtype=np.float32) * 0.05,
        "dot_weights": rng.standard_normal((D, D), dtype=np.float32) * 0.05,
        "mask": np.ones((B, C, Q), dtype=bool),
    }
    out = kernel(**inputs)
    print("out", out.shape, out.dtype)


# revision 20
# speedup vs baseline: 25.4110x; 25.4110x over previous
"""ContextQueryAttention (BiDAF-style) Trainium2 kernel, v7.

Problem: nn_ContextQueryAttention_44066364457466
  query [B=8, Q=512, D=512], context [B=8, C=2048, D=512],
  query_weights/context_weights [D,1], dot_weights [D,D], mask all-True.
  out [B, C, 4D]: concat(context, c2q@query, context*that, context*qtc)

Sharding: data-parallel over batch. B == 8 == n_cores, one batch element
per NeuronCore, no collectives.

Math (per batch element; mask all-ones so it drops out):
  H[d,q]  = sum_e W[d,e] qT[e,q]       (contract the small side first:
  sim[c,q]= sum_d ctx[c,d] H[d,q] + cw[c] + qw[q]   saves 400M MACs vs
                                                    the (ctx@W)@qT order)
  Two-pass softmax around one global shift K = max sim:
    pass 1: evict raw sim to SBUF (f32), row maxes m_c on the fly
    K = max_c m_c  (tiny)
    pass 2: F = exp(sim - K + 60), row sums s_col via the accumulator.
    The +60 keeps the worst row sum (~e^{-90+60}) far enough above the
    f32 floor that 1/s_col cannot overflow; the shift cancels in both
    softmax normalizations.
  Both softmaxes come from this single F:
    c2q = F / rowsum(F)      q2c = F / colsum(F)
  ctq = c2q @ query;  G[q,d] = q2c^T @ ctx;  qtc = c2q @ G

Precision split: the sim chain (transposes, H, sim) runs f32r - exp is
exponentially sensitive to absolute logit error. Everything downstream
of exp works on attention weights where bf16's 0.4% relative error
washes out across 512-2048-term reductions: F, ET, G, and the
ctx/query copies feeding those matmuls are bf16 (the PE rejects
mixed-dtype operands, so rhs-side bf16 casts are made once per body).

Pipelined emission: each body's w/q/cw/qw loads are EMITTED midway
through the previous body (and ctx loads at its end), with a prologue
before the repeat loop, so on the input DMA queue they sit ahead of
the previous body's remaining traffic and their data is resident when
the PE crosses the body boundary. The rot8 pool cycles [wT, qT, H,
w(next)] per body - an even 64 allocations per 16-body loop iteration,
so slot phase is loop-invariant.

Other implementation notes:
- f32r via bitcast for the sim-side matmuls (PE rounds fp32 on ingest).
- Four 128x128 transposes are packed per PSUM bank, one wide eviction.
- cw enters sim through the H eviction bias; qw through one K=1 rank-1
  matmul per c-tile; S_q comes from DVE reduces of the evicted ET.
- DMA queues: sync = inputs + context output copies (dependency-free),
  scalar = ctq block (self-ordered behind its producing mul),
  gpsimd = the two ctx* blocks.
- G has its own pool so next-body wT transposes reuse H's slot (freed
  mid-body) instead of G's (freed only at body end).
- Constants are emitted once, outside the repeat loop.
"""

import numpy as np

B, Q, C, D = 8, 512, 2048, 512
P = 128
QT, CT, DT = Q // P, C // P, D // P  # 4, 16, 4
N_CORES = 8

_NC_CACHE = {}


def ds(start, size):
    return slice(start, start + size)


def _emit_consts(nc, constp):
    import concourse.mybir as mybir

    f32 = mybir.dt.float32
    f32r = mybir.dt.float32r
    bf16 = mybir.dt.bfloat16

    id_f = constp.tile([P, P], f32, name="id_f", tag="id_f")
    from concourse.masks import make_identity
    make_identity(nc, id_f)
    id_r = constp.tile([P, P], f32r, name="id_r", tag="id_r")
    nc.vector.tensor_copy(id_r, id_f)
    id_b = constp.tile([P, P], bf16, name="id_b", tag="id_b")
    nc.vector.tensor_copy(id_b, id_f)
    ones2_f = constp.tile([1, 2], f32, name="ones2_f", tag="ones2_f")
    nc.vector.memset(ones2_f, 1.0)
    ones_row_f = constp.tile([1, 512], f32, name="ones_row_f", tag="ones_row_f")
    nc.vector.memset(ones_row_f, 1.0)
    ones_row_r = constp.tile([1, 512], f32r, name="ones_row", tag="ones_row")
    nc.vector.tensor_copy(ones_row_r, ones_row_f)
    return (id_f, id_r, id_b, ones2_f, ones_row_r)


def _emit_wq_loads(nc, pools, aps):
    """Loads for ONE body's small inputs (w, q, cw, qw). Emitted midway
    through the PREVIOUS body so the data is resident at its start."""
    import concourse.mybir as mybir

    f32r = mybir.dt.float32r
    (statp, ctxps, cbfp, qfam, qbfp, rot8, gpool, fpool, simp, cTp, ETp,
     stagep, ps_mm, ps_tr, ps_st) = pools
    (q_r3, c_r3, w_r3, cw_r3, qw_r3, out_r3) = aps

    w_r = rot8.tile([P, DT, D], f32r, name="w_f", tag="r8")
    nc.sync.dma_start(w_r, w_r3)
    q_r = qfam.tile([P, QT, D], f32r, name="q_f", tag="q_f")
    nc.sync.dma_start(q_r, q_r3)
    cwqw_r = statp.tile([P, DT, 2], f32r, name="cwqw_f", tag="cwqw_f")
    nc.sync.dma_start(cwqw_r[:, :, 0:1], cw_r3)
    nc.sync.dma_start(cwqw_r[:, :, 1:2], qw_r3)
    return {"w_r": w_r, "q_r": q_r, "cwqw_r": cwqw_r}


def _emit_ctx_load(nc, pools, aps):
    """One body's ctx load; emitted at the END of the previous body
    (its dispatch is WAR-blocked on that body's last ctx readers)."""
    import concourse.mybir as mybir

    f32r = mybir.dt.float32r
    (statp, ctxps, cbfp, qfam, qbfp, rot8, gpool, fpool, simp, cTp, ETp,
     stagep, ps_mm, ps_tr, ps_st) = pools
    (q_r3, c_r3, w_r3, cw_r3, qw_r3, out_r3) = aps

    ctx_r = ctxps[0].tile([P, CT, D], f32r, name="ctx_f", tag="ctx_f")
    for g in range(4):
        nc.sync.dma_start(ctx_r[:, ds(g * 4, 4), :], c_r3[:, ds(g * 4, 4), :])
    return ctx_r


def _emit_body(nc, tc, pools, aps, consts, loads, ctx_r, emit_next=True):
    """One body. `loads`/`ctx_r` were emitted during the previous body
    (or are None for the first body of a loop iteration, which loads its
    own). Emits the NEXT body's loads at interleave points unless
    emit_next is False (last body of an iteration)."""
    import concourse.mybir as mybir

    f32 = mybir.dt.float32
    f32r = mybir.dt.float32r
    bf16 = mybir.dt.bfloat16
    Exp = mybir.ActivationFunctionType.Exp
    Copy = mybir.ActivationFunctionType.Copy
    Mult = mybir.AluOpType.mult
    Max = mybir.AluOpType.max
    Min = mybir.AluOpType.min
    AxX = mybir.AxisListType.X

    (statp, ctxps, cbfp, qfam, qbfp, rot8, gpool, fpool, simp, cTp, ETp,
     stagep, ps_mm, ps_tr, ps_st) = pools
    (q_r3, c_r3, w_r3, cw_r3, qw_r3, out_r3) = aps
    (id_f, id_r, id_b, ones2_f, ones_row_r) = consts

    if loads is None:
        loads = _emit_wq_loads(nc, pools, aps)
    if ctx_r is None:
        ctx_r = _emit_ctx_load(nc, pools, aps)
    w_r = loads["w_r"]
    q_r = loads["q_r"]
    cwqw_r = loads["cwqw_r"]
    cwqw_f = cwqw_r.bitcast(f32)
    ctx_f = ctx_r.bitcast(f32)

    # bf16 copies feeding the post-softmax matmul rhs sides
    q_bf = qbfp.tile([P, QT, D], bf16, name="q_bf", tag="q_bf")
    nc.vector.tensor_copy(q_bf, q_r.bitcast(f32))
    ctx_bf = cbfp.tile([P, CT, D], bf16, name="ctx_bf", tag="ctx_bf")
    for g in range(4):
        nc.vector.tensor_copy(ctx_bf[:, ds(g * 4, 4), :],
                              ctx_f[:, ds(g * 4, 4), :])

    def packed_transpose(dst_ap, srcs, evict_engine, ident=id_r, psdt=f32r):
        """len(srcs) transposes into one PSUM bank, one wide eviction."""
        n = len(srcs)
        ps = ps_tr.tile([P, 512], psdt, name="ptr", tag="tr")
        for j, src in enumerate(srcs):
            nc.tensor.matmul(ps[:, ds(j * P, P)], src, ident,
                             is_transpose=True, start=(j == 0),
                             stop=(j == n - 1))
        if evict_engine == "act":
            nc.scalar.copy(dst_ap, ps[:, 0: n * P])
        else:
            nc.vector.tensor_copy(dst_ap, ps[:, 0: n * P])

    # ---- wT [e,d] and qT [e,q] ----
    wT = rot8.tile([P, DT, D], f32r, name="wT", tag="r8")
    for eb in range(DT):
        packed_transpose(
            wT[:, eb, :],
            [w_r[:, dt, ds(eb * P, P)] for dt in range(DT)], "act")
    qT = rot8.tile([P, DT, Q], f32r, name="qT", tag="r8")
    for eb in range(DT):
        packed_transpose(
            qT[:, eb, :],
            [q_r[:, qt, ds(eb * P, P)] for qt in range(QT)], "act")

    # ---- H[d,q] = sum_e wT[e,d] qT[e,q]  (the small D x Q product) ----
    H = rot8.tile([P, DT, Q], f32r, name="H", tag="r8")
    for dtile in range(DT):
        pm = ps_mm.tile([P, 512], f32, name="pm", tag="mm")
        for eb in range(DT):
            nc.tensor.matmul(pm, wT[:, eb, ds(dtile * P, P)], qT[:, eb, :],
                             start=(eb == 0), stop=(eb == DT - 1))
        # H' = H + cw_w[d]: folds the cw[c] similarity term into the
        # contraction (sum_d ctx[c,d] cw_w[d] = cw[c]) at zero extra cost
        nc.scalar.activation(H[:, dtile, :], pm,
                             mybir.ActivationFunctionType.Identity,
                             bias=cwqw_f[:, dtile, 0:1], scale=1.0)

    # ---- qw_row [1, Q] ----
    qw_row = statp.tile([1, Q], f32r, name="qw_row", tag="qw_row")
    pqw = ps_st.tile([1, Q], f32, name="pst", tag="st")
    for dt in range(DT):
        nc.tensor.matmul(pqw, cwqw_r[:, dt, 1:2], qT[:, dt, :],
                         start=(dt == 0), stop=(dt == DT - 1))
    nc.vector.tensor_copy(qw_row, pqw)

    # ---- per ctx chunk: cT transposes, then raw sim for its cts ----
    cT = cTp.tile([P, DT, C], f32r, name="cT", tag="cT")
    sim_r = simp.tile([P, CT, Q], f32, name="sim_t", tag="sim_t")
    F_t = fpool.tile([P, CT, Q], bf16, name="F_t", tag="F_t")
    negm = statp.tile([P, CT], f32, name="negm", tag="negm")
    s_col = statp.tile([P, CT], f32, name="s_col", tag="s_col")

    def emit_cT_chunk(g):
        for dt in range(DT):
            packed_transpose(
                cT[:, dt, ds(g * 512, 512)],
                [ctx_r[:, 4 * g + j, ds(dt * P, P)] for j in range(4)], "dve")

    def emit_sim_chunk(g):
        # the exact-context output block goes straight from SBUF, one
        # dependency-free chunk-sized DMA instead of four per-tile ones
        nc.sync.dma_start(out_r3[:, ds(4 * g, 4), 0:D],
                          ctx_f[:, ds(4 * g, 4), :])
        for ct in range(4 * g, 4 * g + 4):
            pm = ps_mm.tile([P, 512], f32, name="pm", tag="mm")
            for dt in range(DT):
                nc.tensor.matmul(pm, cT[:, dt, ds(ct * P, P)], H[:, dt, :],
                                 start=(dt == 0), stop=False)
            nc.tensor.matmul(pm, ones_row_r[0:1, 0:P], qw_row[0:1, :],
                             start=False, stop=True)
            nc.vector.tensor_reduce(negm[:, ds(ct, 1)], pm, axis=AxX,
                                    op=Max, negate=True)
            nc.scalar.copy(sim_r[:, ct, :], pm)

    # transposes of chunk g+1 are emitted before sim of chunk g so the PE
    # never catches up with the ctx input DMA
    emit_cT_chunk(0)
    emit_cT_chunk(1)
    emit_sim_chunk(0)
    # next body's small loads: the sync queue has drained this body's
    # ctx loads + early ctx-outs by here, and the rot8/qfam slots these
    # overwrite (previous qT / q) are already consumed
    next_loads = _emit_wq_loads(nc, pools, aps) if emit_next else None
    emit_cT_chunk(2)
    emit_sim_chunk(1)
    emit_cT_chunk(3)
    emit_sim_chunk(2)
    emit_sim_chunk(3)

    # ---- global max K; negK_col = (60 - K) broadcast down partitions ----
    nkp = statp.tile([P, 1], f32, name="nkp", tag="nkp")
    nc.vector.tensor_reduce(nkp, negm, axis=AxX, op=Min)  # -max_q per row
    pkt = ps_st.tile([1, P], f32, name="pst", tag="st")
    nc.tensor.transpose(pkt, nkp, id_f)
    negK11 = statp.tile([1, 1], f32, name="negK11", tag="negK11")
    nc.vector.tensor_reduce(negK11, pkt, axis=AxX, op=Min)  # -K
    # shift by +60: F = exp(sim - K + 60). The shift cancels in both
    # softmax normalizations but keeps the worst row sum (~e^{-90+60})
    # far from the f32 range floor, so 1/s_col cannot overflow to inf.
    krow = statp.tile([1, P], f32, name="krow", tag="krow")
    nc.vector.tensor_scalar_add(krow, negK11.to_broadcast([1, P]), 60.0)
    pkb = ps_st.tile([P, 2], f32, name="pst", tag="st")
    nc.tensor.matmul(pkb, krow, ones2_f, start=True, stop=True)
    negK_col = statp.tile([P, 1], f32, name="negK_col", tag="negK_col")
    nc.vector.tensor_copy(negK_col, pkb[:, 0:1])

    # ---- F = exp(sim - K + 60) (bf16); ET [q,c] = F^T; S_q by DVE
    # reduces of the evicted ET. exp of chunk g and ET of chunk g are
    # interleaved so the PE restarts on the first four F tiles. ----
    ET = ETp.tile([P, QT, C], bf16, name="ET", tag="ET")
    sqp = statp.tile([P, QT, 4], f32, name="sqp", tag="sqp")
    for g in range(4):
        for ct in range(4 * g, 4 * g + 4):
            # scale=1.0 only: negative activation scale miscomputes on HW
            nc.scalar.activation(F_t[:, ct, :], sim_r[:, ct, :], Exp,
                                 bias=negK_col, scale=1.0,
                                 accum_out=s_col[:, ds(ct, 1)])
        for qt in range(QT):
            packed_transpose(
                ET[:, qt, ds(g * 512, 512)],
                [F_t[:, 4 * g + j, ds(qt * P, P)] for j in range(4)],
                "dve", ident=id_b, psdt=bf16)
            nc.vector.tensor_reduce(sqp[:, qt, ds(g, 1)],
                                    ET[:, qt, ds(g * 512, 512)], axis=AxX,
                                    op=mybir.AluOpType.add)
    r_col = statp.tile([P, CT], f32, name="r_col", tag="r_col")
    nc.vector.reciprocal(r_col, s_col)
    sq = statp.tile([P, QT], f32, name="sq", tag="sq")
    for qt in range(QT):
        nc.vector.tensor_reduce(sq[:, ds(qt, 1)], sqp[:, qt, :], axis=AxX,
                                op=mybir.AluOpType.add)
    rq = statp.tile([P, QT], f32, name="rq", tag="rq")
    nc.vector.reciprocal(rq, sq)

    # ---- G[q,d] = (1/S_q) sum_c F[c,q] ctx[c,d] ----
    G_b = gpool.tile([P, QT, D], bf16, name="G_b", tag="G_b")
    for qt in range(QT):
        pm = ps_mm.tile([P, 512], f32, name="pm", tag="mm")
        for ct in range(CT):
            nc.tensor.matmul(pm, F_t[:, ct, ds(qt * P, P)],
                             ctx_bf[:, ct, :],
                             start=(ct == 0), stop=(ct == CT - 1))
        nc.scalar.mul(G_b[:, qt, :], pm, rq[:, ds(qt, 1)])

    # ---- outputs per ct ----
    for ct in range(CT):
        pc = ps_mm.tile([P, 512], f32, name="pm", tag="mm")
        for qt in range(QT):
            nc.tensor.matmul(pc, ET[:, qt, ds(ct * P, P)], q_bf[:, qt, :],
                             start=(qt == 0), stop=(qt == QT - 1))
        pq = ps_mm.tile([P, 512], f32, name="pm", tag="mm")
        for qt in range(QT):
            nc.tensor.matmul(pq, ET[:, qt, ds(ct * P, P)], G_b[:, qt, :],
                             start=(qt == 0), stop=(qt == QT - 1))
        st = stagep.tile([P, 3 * D], f32, name="st", tag="stage")
        # three writers on independent engines; scalar DMAs the block it
        # produced itself (self-ordered), gpsimd takes the DVE blocks
        nc.scalar.mul(st[:, 0:D], pc, r_col[:, ds(ct, 1)])
        nc.scalar.dma_start(out_r3[:, ct, ds(D, D)], st[:, 0:D])
        ctx_slice = ctx_bf[:, ct, :]
        nc.vector.scalar_tensor_tensor(st[:, ds(D, D)], pc,
                                       r_col[:, ds(ct, 1)], ctx_slice,
                                       op0=Mult, op1=Mult)
        nc.vector.scalar_tensor_tensor(st[:, ds(2 * D, D)], pq,
                                       r_col[:, ds(ct, 1)], ctx_slice,
                                       op0=Mult, op1=Mult)
        nc.gpsimd.dma_start(out_r3[:, ct, ds(2 * D, 2 * D)], st[:, ds(D, 2 * D)])

    # next body's ctx load: emitted last - its dispatch is WAR-blocked on
    # this body's final ctx_f readers (the scalar_tensor_tensor ops above)
    next_ctx = _emit_ctx_load(nc, pools, aps) if emit_next else None
    return next_loads, next_ctx


def _build_bass(loop_n=1):
    import concourse.bass as bass  # noqa: F401
    import concourse.mybir as mybir
    import concourse.tile as tile
    from concourse import bacc

    f32 = mybir.dt.float32

    f32r = mybir.dt.float32r
    nc = bacc.Bacc("TRN2", debug=False, num_devices=N_CORES)
    q_d = nc.dram_tensor("query", [Q, D], f32r, kind="ExternalInput")
    c_d = nc.dram_tensor("context", [C, D], f32r, kind="ExternalInput")
    qw_d = nc.dram_tensor("query_weights", [D, 1], f32r, kind="ExternalInput")
    cw_d = nc.dram_tensor("context_weights", [D, 1], f32r, kind="ExternalInput")
    w_d = nc.dram_tensor("dot_weights", [D, D], f32r, kind="ExternalInput")
    out_d = nc.dram_tensor("out", [C, 4 * D], f32, kind="ExternalOutput")

    aps = (
        q_d.ap().rearrange("(t p) d -> p t d", p=P),
        c_d.ap().rearrange("(t p) d -> p t d", p=P),
        w_d.ap().rearrange("(t p) e -> p t e", p=P),
        cw_d.ap().rearrange("(t p) o -> p t o", p=P),
        qw_d.ap().rearrange("(t p) o -> p t o", p=P),
        out_d.ap().rearrange("(t p) f -> p t f", p=P),
    )

    with tile.TileContext(nc) as tc:
        from contextlib import ExitStack
        with ExitStack() as es:
            def pool(name, bufs, space="SBUF"):
                return es.enter_context(
                    tc.tile_pool(name=name, bufs=bufs, space=space))

            constp = pool("const", 1)
            statp = pool("stats", 1)
            ctx0p = pool("ctx0", 1)
            ctx1p = pool("ctx1", 1)
            ctx2p = pool("ctx2", 1)
            ctx3p = pool("ctx3", 1)
            cbfp = pool("cbf", 1)
            qfam = pool("qfam", 1)
            qbfp = pool("qbf", 1)
            rot8 = pool("rot8", 2)
            gpool = pool("gpool", 2)
            fpool = pool("fpool", 1)
            simp = pool("simp", 1)
            cTp = pool("cTp", 1)
            ETp = pool("ETp", 1)
            stagep = pool("stage", 3)
            ps_mm = pool("ps_mm", 4, space="PSUM")
            ps_tr = pool("ps_tr", 3, space="PSUM")
            ps_st = pool("ps_st", 1, space="PSUM")

            pools = (statp, (ctx0p, ctx1p, ctx2p, ctx3p), cbfp, qfam,
                     qbfp, rot8, gpool, fpool, simp, cTp, ETp, stagep,
                     ps_mm, ps_tr, ps_st)
            consts = _emit_consts(nc, constp)
            if loop_n > 1:
                # unroll several bodies per loop iteration: the For_i
                # all-engine barrier is expensive on this runtime, so
                # amortize it while keeping exactly loop_n body runs.
                # Body 0 of each iteration loads its own inputs (tiles
                # allocated outside For_i are never released inside it,
                # so a prologue would deadlock the pool rotation).
                k = 16 if loop_n % 16 == 0 else (
                    8 if loop_n % 8 == 0 else (4 if loop_n % 4 == 0 else 1))
                with tc.For_i(0, loop_n // k, 1):
                    loads = ctx_r = None
                    for b in range(k):
                        loads, ctx_r = _emit_body(nc, tc, pools, aps,
                                                  consts, loads, ctx_r,
                                                  emit_next=(b < k - 1))
            else:
                _emit_body(nc, tc, pools, aps, consts, None, None,
                           emit_next=False)
    nc.compile()
    return nc


def get_nc(loop_n=1):
    if loop_n not in _NC_CACHE:
        _NC_CACHE[loop_n] = _build_bass(loop_n)
    return _NC_CACHE[loop_n]


def kernel(query, context, query_weights, context_weights, dot_weights,
           mask=None):
    from concourse.bass_utils import run_bass_kernel_spmd

    query = np.ascontiguousarray(np.asarray(query, dtype=np.float32))
    context = np.ascontiguousarray(np.asarray(context, dtype=np.float32))
    query_weights = np.ascontiguousarray(np.asarray(query_weights, dtype=np.float32))
    context_weights = np.ascontiguousarray(np.asarray(context_weights, dtype=np.float32))
    dot_weights = np.ascontiguousarray(np.asarray(dot_weights, dtype=np.float32))
    # mask is all-True per the problem spec; NEG_INF * (~mask) == 0, so it
    # drops out of the computation entirely.

    nc = get_nc()
    in_maps = [
        {
            "query": query[b],
            "context": context[b],
            "query_weights": query_weights,
            "context_weights": context_weights,
            "dot_weights": dot_weights,
        }
        for b in range(B)
    ]
    res = run_bass_kernel_spmd(nc, in_maps, core_ids=list(range(N_CORES)))
    out = np.stack([res.results[b]["out"] for b in range(B)], axis=0)
    return np.ascontiguousarray(out.astype(np.float32))


if __name__ == "__main__":
    rng = np.random.default_rng(0)
    inputs = {
        "query": rng.standard_normal((B, Q, D), dtype=np.float32),
        "context": rng.standard_normal((B, C, D), dtype=np.float32),
        "query_weights": rng.standard_normal((D, 1), dtype=np.float32) * 0.05,
        "context_weights": rng.standard_normal((D, 1), dtype=np.float32) * 0.05,
        "dot_weights": rng.standard_normal((D, D), dtype=np.float32) * 0.05,
        "mask": np.ones((B, C, Q), dtype=bool),
    }
    out = kernel(**inputs)
    print("out", out.shape, out.dtype)


# revision 21
# speedup vs baseline: 26.7497x; 1.0527x over previous
"""ContextQueryAttention (BiDAF-style) Trainium2 kernel, v7.

Problem: nn_ContextQueryAttention_44066364457466
  query [B=8, Q=512, D=512], context [B=8, C=2048, D=512],
  query_weights/context_weights [D,1], dot_weights [D,D], mask all-True.
  out [B, C, 4D]: concat(context, c2q@query, context*that, context*qtc)

Sharding: data-parallel over batch. B == 8 == n_cores, one batch element
per NeuronCore, no collectives.

Math (per batch element; mask all-ones so it drops out):
  H[d,q]  = sum_e W[d,e] qT[e,q]       (contract the small side first:
  sim[c,q]= sum_d ctx[c,d] H[d,q] + cw[c] + qw[q]   saves 400M MACs vs
                                                    the (ctx@W)@qT order)
  Two-pass softmax around one global shift K = max sim:
    pass 1: evict raw sim to SBUF (f32), row maxes m_c on the fly
    K = max_c m_c  (tiny)
    pass 2: F = exp(sim - K + 60), row sums s_col via the accumulator.
    The +60 keeps the worst row sum (~e^{-90+60}) far enough above the
    f32 floor that 1/s_col cannot overflow; the shift cancels in both
    softmax normalizations.
  Both softmaxes come from this single F:
    c2q = F / rowsum(F)      q2c = F / colsum(F)
  ctq = c2q @ query;  G[q,d] = q2c^T @ ctx;  qtc = c2q @ G

Precision split: the sim chain (transposes, H, sim) runs f32r - exp is
exponentially sensitive to absolute logit error. Everything downstream
of exp works on attention weights where bf16's 0.4% relative error
washes out across 512-2048-term reductions: F, ET, G, and the
ctx/query copies feeding those matmuls are bf16 (the PE rejects
mixed-dtype operands, so rhs-side bf16 casts are made once per body).

Pipelined emission: each body's w/q/cw/qw loads are EMITTED midway
through the previous body (and ctx loads at its end), with a prologue
before the repeat loop, so on the input DMA queue they sit ahead of
the previous body's remaining traffic and their data is resident when
the PE crosses the body boundary. The rot8 pool cycles [wT, qT, H,
w(next)] per body - an even 64 allocations per 16-body loop iteration,
so slot phase is loop-invariant.

Other implementation notes:
- f32r via bitcast for the sim-side matmuls (PE rounds fp32 on ingest).
- Four 128x128 transposes are packed per PSUM bank, one wide eviction.
- cw enters sim through the H eviction bias; qw through one K=1 rank-1
  matmul per c-tile; S_q comes from DVE reduces of the evicted ET.
- DMA queues: sync = inputs + context output copies (dependency-free),
  scalar = ctq block (self-ordered behind its producing mul),
  gpsimd = the two ctx* blocks.
- G has its own pool so next-body wT transposes reuse H's slot (freed
  mid-body) instead of G's (freed only at body end).
- Constants are emitted once, outside the repeat loop.
"""

import numpy as np

B, Q, C, D = 8, 512, 2048, 512
P = 128
QT, CT, DT = Q // P, C // P, D // P  # 4, 16, 4
N_CORES = 8

_NC_CACHE = {}


def ds(start, size):
    return slice(start, start + size)


def _emit_consts(nc, constp):
    import concourse.mybir as mybir

    f32 = mybir.dt.float32
    f32r = mybir.dt.float32r
    bf16 = mybir.dt.bfloat16

    id_f = constp.tile([P, P], f32, name="id_f", tag="id_f")
    from concourse.masks import make_identity
    make_identity(nc, id_f)
    id_r = constp.tile([P, P], f32r, name="id_r", tag="id_r")
    nc.vector.tensor_copy(id_r, id_f)
    id_b = constp.tile([P, P], bf16, name="id_b", tag="id_b")
    nc.vector.tensor_copy(id_b, id_f)
    ones2_f = constp.tile([1, 2], f32, name="ones2_f", tag="ones2_f")
    nc.vector.memset(ones2_f, 1.0)
    ones_row_f = constp.tile([1, 512], f32, name="ones_row_f", tag="ones_row_f")
    nc.vector.memset(ones_row_f, 1.0)
    ones_row_r = constp.tile([1, 512], f32r, name="ones_row", tag="ones_row")
    nc.vector.tensor_copy(ones_row_r, ones_row_f)
    return (id_f, id_r, id_b, ones2_f, ones_row_r)


def _emit_wq_loads(nc, pools, aps):
    """Loads for ONE body's small inputs (w, q, cw, qw). Emitted midway
    through the PREVIOUS body so the data is resident at its start."""
    import concourse.mybir as mybir

    f32r = mybir.dt.float32r
    (statp, ctxps, cbfp, qfam, qbfp, rot8, gpool, fpool, simp, cTp, ETp,
     stagep, ps_mm, ps_tr, ps_st) = pools
    (q_r3, c_r3, w_r3, cw_r3, qw_r3, out_r3) = aps

    w_r = rot8.tile([P, DT, D], f32r, name="w_f", tag="r8")
    nc.sync.dma_start(w_r, w_r3)
    q_r = qfam.tile([P, QT, D], f32r, name="q_f", tag="q_f")
    nc.sync.dma_start(q_r, q_r3)
    cwqw_r = statp.tile([P, DT, 2], f32r, name="cwqw_f", tag="cwqw_f")
    nc.sync.dma_start(cwqw_r[:, :, 0:1], cw_r3)
    nc.sync.dma_start(cwqw_r[:, :, 1:2], qw_r3)
    return {"w_r": w_r, "q_r": q_r, "cwqw_r": cwqw_r}


def _emit_ctx_load(nc, pools, aps):
    """One body's ctx load; emitted at the END of the previous body
    (its dispatch is WAR-blocked on that body's last ctx readers)."""
    import concourse.mybir as mybir

    f32r = mybir.dt.float32r
    (statp, ctxps, cbfp, qfam, qbfp, rot8, gpool, fpool, simp, cTp, ETp,
     stagep, ps_mm, ps_tr, ps_st) = pools
    (q_r3, c_r3, w_r3, cw_r3, qw_r3, out_r3) = aps

    ctx_r = ctxps[0].tile([P, CT, D], f32r, name="ctx_f", tag="ctx_f")
    for g in range(4):
        nc.sync.dma_start(ctx_r[:, ds(g * 4, 4), :], c_r3[:, ds(g * 4, 4), :])
    return ctx_r


def _emit_body(nc, tc, pools, aps, consts, loads, ctx_r, emit_next=True):
    """One body. `loads`/`ctx_r` were emitted during the previous body
    (or are None for the first body of a loop iteration, which loads its
    own). Emits the NEXT body's loads at interleave points unless
    emit_next is False (last body of an iteration)."""
    import concourse.mybir as mybir

    f32 = mybir.dt.float32
    f32r = mybir.dt.float32r
    bf16 = mybir.dt.bfloat16
    Exp = mybir.ActivationFunctionType.Exp
    Copy = mybir.ActivationFunctionType.Copy
    Mult = mybir.AluOpType.mult
    Max = mybir.AluOpType.max
    Min = mybir.AluOpType.min
    AxX = mybir.AxisListType.X

    (statp, ctxps, cbfp, qfam, qbfp, rot8, gpool, fpool, simp, cTp, ETp,
     stagep, ps_mm, ps_tr, ps_st) = pools
    (q_r3, c_r3, w_r3, cw_r3, qw_r3, out_r3) = aps
    (id_f, id_r, id_b, ones2_f, ones_row_r) = consts

    if loads is None:
        loads = _emit_wq_loads(nc, pools, aps)
    if ctx_r is None:
        ctx_r = _emit_ctx_load(nc, pools, aps)
    w_r = loads["w_r"]
    q_r = loads["q_r"]
    cwqw_r = loads["cwqw_r"]
    cwqw_f = cwqw_r.bitcast(f32)
    ctx_f = ctx_r.bitcast(f32)

    # bf16 copies feeding the post-softmax matmul rhs sides
    q_bf = qbfp.tile([P, QT, D], bf16, name="q_bf", tag="q_bf")
    nc.vector.tensor_copy(q_bf, q_r.bitcast(f32))
    ctx_bf = cbfp.tile([P, CT, D], bf16, name="ctx_bf", tag="ctx_bf")
    for g in range(4):
        nc.vector.tensor_copy(ctx_bf[:, ds(g * 4, 4), :],
                              ctx_f[:, ds(g * 4, 4), :])

    def packed_transpose(dst_ap, srcs, evict_engine, ident=id_r, psdt=f32r):
        """len(srcs) transposes into one PSUM bank, one wide eviction."""
        n = len(srcs)
        ps = ps_tr.tile([P, 512], psdt, name="ptr", tag="tr")
        for j, src in enumerate(srcs):
            nc.tensor.matmul(ps[:, ds(j * P, P)], src, ident,
                             is_transpose=True, start=(j == 0),
                             stop=(j == n - 1))
        if evict_engine == "act":
            nc.scalar.copy(dst_ap, ps[:, 0: n * P])
        else:
            nc.vector.tensor_copy(dst_ap, ps[:, 0: n * P])

    # ---- wT [e,d] and qT [e,q]: carried from the previous body's
    # K-barrier emission, or built inline for an iteration's first body ----
    def emit_wq_transposes(wsrc, qsrc):
        wT_ = rot8.tile([P, DT, D], f32r, name="wT", tag="r8")
        for eb in range(DT):
            packed_transpose(
                wT_[:, eb, :],
                [wsrc[:, dt, ds(eb * P, P)] for dt in range(DT)], "act")
        qT_ = rot8.tile([P, DT, Q], f32r, name="qT", tag="r8")
        for eb in range(DT):
            packed_transpose(
                qT_[:, eb, :],
                [qsrc[:, qt, ds(eb * P, P)] for qt in range(QT)], "act")
        return wT_, qT_

    if "wT" in loads:
        wT, qT = loads["wT"], loads["qT"]
    else:
        wT, qT = emit_wq_transposes(w_r, q_r)

    # ---- H[d,q] = sum_e wT[e,d] qT[e,q]  (the small D x Q product) ----
    H = rot8.tile([P, DT, Q], f32r, name="H", tag="r8")
    for dtile in range(DT):
        pm = ps_mm.tile([P, 512], f32, name="pm", tag="mm")
        for eb in range(DT):
            nc.tensor.matmul(pm, wT[:, eb, ds(dtile * P, P)], qT[:, eb, :],
                             start=(eb == 0), stop=(eb == DT - 1))
        # H' = H + cw_w[d]: folds the cw[c] similarity term into the
        # contraction (sum_d ctx[c,d] cw_w[d] = cw[c]) at zero extra cost
        nc.scalar.activation(H[:, dtile, :], pm,
                             mybir.ActivationFunctionType.Identity,
                             bias=cwqw_f[:, dtile, 0:1], scale=1.0)

    # ---- qw_row [1, Q] ----
    qw_row = statp.tile([1, Q], f32r, name="qw_row", tag="qw_row")
    pqw = ps_st.tile([1, Q], f32, name="pst", tag="st")
    for dt in range(DT):
        nc.tensor.matmul(pqw, cwqw_r[:, dt, 1:2], qT[:, dt, :],
                         start=(dt == 0), stop=(dt == DT - 1))
    nc.vector.tensor_copy(qw_row, pqw)

    # ---- per ctx chunk: cT transposes, then raw sim for its cts ----
    cT = cTp.tile([P, DT, C], f32r, name="cT", tag="cT")
    sim_r = simp.tile([P, CT, Q], f32, name="sim_t", tag="sim_t")
    F_t = fpool.tile([P, CT, Q], bf16, name="F_t", tag="F_t")
    negm = statp.tile([P, CT], f32, name="negm", tag="negm")
    s_col = statp.tile([P, CT], f32, name="s_col", tag="s_col")

    def emit_cT_chunk(g):
        for dt in range(DT):
            packed_transpose(
                cT[:, dt, ds(g * 512, 512)],
                [ctx_r[:, 4 * g + j, ds(dt * P, P)] for j in range(4)], "dve")

    def emit_sim_chunk(g):
        # the exact-context output block goes straight from SBUF, one
        # dependency-free chunk-sized DMA instead of four per-tile ones
        nc.sync.dma_start(out_r3[:, ds(4 * g, 4), 0:D],
                          ctx_f[:, ds(4 * g, 4), :])
        for ct in range(4 * g, 4 * g + 4):
            pm = ps_mm.tile([P, 512], f32, name="pm", tag="mm")
            for dt in range(DT):
                nc.tensor.matmul(pm, cT[:, dt, ds(ct * P, P)], H[:, dt, :],
                                 start=(dt == 0), stop=False)
            nc.tensor.matmul(pm, ones_row_r[0:1, 0:P], qw_row[0:1, :],
                             start=False, stop=True)
            nc.vector.tensor_reduce(negm[:, ds(ct, 1)], pm, axis=AxX,
                                    op=Max, negate=True)
            nc.scalar.copy(sim_r[:, ct, :], pm)

    # transposes of chunk g+1 are emitted before sim of chunk g so the PE
    # never catches up with the ctx input DMA
    emit_cT_chunk(0)
    emit_cT_chunk(1)
    emit_sim_chunk(0)
    # next body's small loads: the sync queue has drained this body's
    # ctx loads + early ctx-outs by here, and the rot8/qfam slots these
    # overwrite (previous qT / q) are already consumed
    next_loads = _emit_wq_loads(nc, pools, aps) if emit_next else None
    emit_cT_chunk(2)
    emit_sim_chunk(1)
    emit_cT_chunk(3)
    emit_sim_chunk(2)
    emit_sim_chunk(3)

    # fill the K-barrier PE bubble with the NEXT body's wT/qT transposes
    # (their inputs loaded mid-body; every post-sim op here needs K first)
    if emit_next and next_loads is not None:
        next_loads["wT"], next_loads["qT"] = emit_wq_transposes(
            next_loads["w_r"], next_loads["q_r"])

    # ---- global max K; negK_col = (60 - K) broadcast down partitions ----
    nkp = statp.tile([P, 1], f32, name="nkp", tag="nkp")
    nc.vector.tensor_reduce(nkp, negm, axis=AxX, op=Min)  # -max_q per row
    pkt = ps_st.tile([1, P], f32, name="pst", tag="st")
    nc.tensor.transpose(pkt, nkp, id_f)
    negK11 = statp.tile([1, 1], f32, name="negK11", tag="negK11")
    nc.vector.tensor_reduce(negK11, pkt, axis=AxX, op=Min)  # -K
    # shift by +60: F = exp(sim - K + 60). The shift cancels in both
    # softmax normalizations but keeps the worst row sum (~e^{-90+60})
    # far from the f32 range floor, so 1/s_col cannot overflow to inf.
    krow = statp.tile([1, P], f32, name="krow", tag="krow")
    nc.vector.tensor_scalar_add(krow, negK11.to_broadcast([1, P]), 60.0)
    pkb = ps_st.tile([P, 2], f32, name="pst", tag="st")
    nc.tensor.matmul(pkb, krow, ones2_f, start=True, stop=True)
    negK_col = statp.tile([P, 1], f32, name="negK_col", tag="negK_col")
    nc.vector.tensor_copy(negK_col, pkb[:, 0:1])

    # ---- F = exp(sim - K + 60) (bf16); ET [q,c] = F^T; S_q by DVE
    # reduces of the evicted ET. exp of chunk g and ET of chunk g are
    # interleaved so the PE restarts on the first four F tiles. ----
    ET = ETp.tile([P, QT, C], bf16, name="ET", tag="ET")
    sqp = statp.tile([P, QT, 4], f32, name="sqp", tag="sqp")
    for g in range(4):
        for ct in range(4 * g, 4 * g + 4):
            # scale=1.0 only: negative activation scale miscomputes on HW
            nc.scalar.activation(F_t[:, ct, :], sim_r[:, ct, :], Exp,
                                 bias=negK_col, scale=1.0,
                                 accum_out=s_col[:, ds(ct, 1)])
        for qt in range(QT):
            packed_transpose(
                ET[:, qt, ds(g * 512, 512)],
                [F_t[:, 4 * g + j, ds(qt * P, P)] for j in range(4)],
                "dve", ident=id_b, psdt=bf16)
            nc.vector.tensor_reduce(sqp[:, qt, ds(g, 1)],
                                    ET[:, qt, ds(g * 512, 512)], axis=AxX,
                                    op=mybir.AluOpType.add)
    r_col = statp.tile([P, CT], f32, name="r_col", tag="r_col")
    nc.vector.reciprocal(r_col, s_col)
    sq = statp.tile([P, QT], f32, name="sq", tag="sq")
    for qt in range(QT):
        nc.vector.tensor_reduce(sq[:, ds(qt, 1)], sqp[:, qt, :], axis=AxX,
                                op=mybir.AluOpType.add)
    rq = statp.tile([P, QT], f32, name="rq", tag="rq")
    nc.vector.reciprocal(rq, sq)

    # ---- G[q,d] = (1/S_q) sum_c F[c,q] ctx[c,d] ----
    G_b = gpool.tile([P, QT, D], bf16, name="G_b", tag="G_b")
    for qt in range(QT):
        pm = ps_mm.tile([P, 512], f32, name="pm", tag="mm")
        for ct in range(CT):
            nc.tensor.matmul(pm, F_t[:, ct, ds(qt * P, P)],
                             ctx_bf[:, ct, :],
                             start=(ct == 0), stop=(ct == CT - 1))
        nc.scalar.mul(G_b[:, qt, :], pm, rq[:, ds(qt, 1)])

    # ---- outputs per ct ----
    for ct in range(CT):
        pc = ps_mm.tile([P, 512], f32, name="pm", tag="mm")
        for qt in range(QT):
            nc.tensor.matmul(pc, ET[:, qt, ds(ct * P, P)], q_bf[:, qt, :],
                             start=(qt == 0), stop=(qt == QT - 1))
        pq = ps_mm.tile([P, 512], f32, name="pm", tag="mm")
        for qt in range(QT):
            nc.tensor.matmul(pq, ET[:, qt, ds(ct * P, P)], G_b[:, qt, :],
                             start=(qt == 0), stop=(qt == QT - 1))
        st = stagep.tile([P, 3 * D], f32, name="st", tag="stage")
        # three writers on independent engines; scalar DMAs the block it
        # produced itself (self-ordered), gpsimd takes the DVE blocks
        nc.scalar.mul(st[:, 0:D], pc, r_col[:, ds(ct, 1)])
        nc.scalar.dma_start(out_r3[:, ct, ds(D, D)], st[:, 0:D])
        ctx_slice = ctx_bf[:, ct, :]
        nc.vector.scalar_tensor_tensor(st[:, ds(D, D)], pc,
                                       r_col[:, ds(ct, 1)], ctx_slice,
                                       op0=Mult, op1=Mult)
        nc.vector.scalar_tensor_tensor(st[:, ds(2 * D, D)], pq,
                                       r_col[:, ds(ct, 1)], ctx_slice,
                                       op0=Mult, op1=Mult)
        nc.gpsimd.dma_start(out_r3[:, ct, ds(2 * D, 2 * D)], st[:, ds(D, 2 * D)])

    # next body's ctx load: emitted last - its dispatch is WAR-blocked on
    # this body's final ctx_f readers (the scalar_tensor_tensor ops above)
    next_ctx = _emit_ctx_load(nc, pools, aps) if emit_next else None
    return next_loads, next_ctx


def _build_bass(loop_n=1):
    import concourse.bass as bass  # noqa: F401
    import concourse.mybir as mybir
    import concourse.tile as tile
    from concourse import bacc

    f32 = mybir.dt.float32

    f32r = mybir.dt.float32r
    nc = bacc.Bacc("TRN2", debug=False, num_devices=N_CORES)
    q_d = nc.dram_tensor("query", [Q, D], f32r, kind="ExternalInput")
    c_d = nc.dram_tensor("context", [C, D], f32r, kind="ExternalInput")
    qw_d = nc.dram_tensor("query_weights", [D, 1], f32r, kind="ExternalInput")
    cw_d = nc.dram_tensor("context_weights", [D, 1], f32r, kind="ExternalInput")
    w_d = nc.dram_tensor("dot_weights", [D, D], f32r, kind="ExternalInput")
    out_d = nc.dram_tensor("out", [C, 4 * D], f32, kind="ExternalOutput")

    aps = (
        q_d.ap().rearrange("(t p) d -> p t d", p=P),
        c_d.ap().rearrange("(t p) d -> p t d", p=P),
        w_d.ap().rearrange("(t p) e -> p t e", p=P),
        cw_d.ap().rearrange("(t p) o -> p t o", p=P),
        qw_d.ap().rearrange("(t p) o -> p t o", p=P),
        out_d.ap().rearrange("(t p) f -> p t f", p=P),
    )

    with tile.TileContext(nc) as tc:
        from contextlib import ExitStack
        with ExitStack() as es:
            def pool(name, bufs, space="SBUF"):
                return es.enter_context(
                    tc.tile_pool(name=name, bufs=bufs, space=space))

            constp = pool("const", 1)
            statp = pool("stats", 1)
            ctx0p = pool("ctx0", 1)
            ctx1p = pool("ctx1", 1)
            ctx2p = pool("ctx2", 1)
            ctx3p = pool("ctx3", 1)
            cbfp = pool("cbf", 1)
            qfam = pool("qfam", 1)
            qbfp = pool("qbf", 1)
            rot8 = pool("rot8", 2)
            gpool = pool("gpool", 2)
            fpool = pool("fpool", 1)
            simp = pool("simp", 1)
            cTp = pool("cTp", 1)
            ETp = pool("ETp", 1)
            stagep = pool("stage", 3)
            ps_mm = pool("ps_mm", 4, space="PSUM")
            ps_tr = pool("ps_tr", 3, space="PSUM")
            ps_st = pool("ps_st", 1, space="PSUM")

            pools = (statp, (ctx0p, ctx1p, ctx2p, ctx3p), cbfp, qfam,
                     qbfp, rot8, gpool, fpool, simp, cTp, ETp, stagep,
                     ps_mm, ps_tr, ps_st)
            consts = _emit_consts(nc, constp)
            if loop_n > 1:
                # unroll several bodies per loop iteration: the For_i
                # all-engine barrier is expensive on this runtime, so
                # amortize it while keeping exactly loop_n body runs.
                # Body 0 of each iteration loads its own inputs (tiles
                # allocated outside For_i are never released inside it,
                # so a prologue would deadlock the pool rotation).
                k = 16 if loop_n % 16 == 0 else (
                    8 if loop_n % 8 == 0 else (4 if loop_n % 4 == 0 else 1))
                with tc.For_i(0, loop_n // k, 1):
                    loads = ctx_r = None
                    for b in range(k):
                        loads, ctx_r = _emit_body(nc, tc, pools, aps,
                                                  consts, loads, ctx_r,
                                                  emit_next=(b < k - 1))
            else:
                _emit_body(nc, tc, pools, aps, consts, None, None,
                           emit_next=False)
    nc.compile()
    return nc


def get_nc(loop_n=1):
    if loop_n not in _NC_CACHE:
        _NC_CACHE[loop_n] = _build_bass(loop_n)
    return _NC_CACHE[loop_n]


def kernel(query, context, query_weights, context_weights, dot_weights,
           mask=None):
    from concourse.bass_utils import run_bass_kernel_spmd

    query = np.ascontiguousarray(np.asarray(query, dtype=np.float32))
    context = np.ascontiguousarray(np.asarray(context, dtype=np.float32))
    query_weights = np.ascontiguousarray(np.asarray(query_weights, dtype=np.float32))
    context_weights = np.ascontiguousarray(np.asarray(context_weights, dtype=np.float32))
    dot_weights = np.ascontiguousarray(np.asarray(dot_weights, dtype=np.float32))
    # mask is all-True per the problem spec; NEG_INF * (~mask) == 0, so it
    # drops out of the computation entirely.

    nc = get_nc()
    in_maps = [
        {
            "query": query[b],
            "context": context[b],
            "query_weights": query_weights,
            "context_weights": context_weights,
            "dot_weights": dot_weights,
        }
        for b in range(B)
    ]
    res = run_bass_kernel_spmd(nc, in_maps, core_ids=list(range(N_CORES)))
    out = np.stack([res.results[b]["out"] for b in range(B)], axis=0)
    return np.ascontiguousarray(out.astype(np.float32))


if __name__ == "__main__":
    rng = np.random.default_rng(0)
    inputs = {
        "query": rng.standard_normal((B, Q, D), dtype=np.float32),
        "context": rng.standard_normal((B, C, D), dtype=np.float32),
        "query_weights": rng.standard_normal((D, 1), dtype=np.float32) * 0.05,
        "context_weights": rng.standard_normal((D, 1), dtype=np.float32) * 0.05,
        "dot_weights": rng.standard_normal((D, D), dtype=np.float32) * 0.05,
        "mask": np.ones((B, C, Q), dtype=bool),
    }
    out = kernel(**inputs)
    print("out", out.shape, out.dtype)
